# revision 9
# baseline (speedup 1.0000x reference)
"""Multi-head self-attention (B=2, N=4096, D=512, H=8, dh=64) on 8 trn2 cores.

Sharding: batch*heads across cores. Core c handles batch b=c//4 and the
head-pair p=c%4 (a 128-wide slice of the inner dim). Each core computes its
two heads' attention and a partial output projection; the host sums the 4
partials per batch and adds the bias.

v2 structure (engines are in-order FIFOs, so emission order is the schedule):
 - scores ST[j, i] as before (kT slab stationary, qT moving, bf16), exp split
   between ScalarE (activation, scale=16 on SCALE/16-pre-scaled scores) and
   VectorE (custom DVE poly op EXP_POLY16_ANT, one instruction per block).
 - AV is FLIPPED: out O[i, d] accumulates with the attention-weight tile as
   the STATIONARY operand (lhsT = et[:, i-block], 128x128) and V (65 cols,
   ones column at 64 for the softmax denominator) as the moving operand.
   The tensor engine charges only the moving side: 65 rows per (i-block, j)
   instead of 512 per (j, chunk) -- AV drops from 262k to 133k PE cycles.
 - softmax normalize: O[i, 64] is the per-partition denominator, so the
   whole normalize is one gpsimd normalize_recip per i-block (no partition
   broadcast, no VectorE reciprocal).
 - normalized rows transpose back (PE, [128,64] -> [64,128]) into OTn fp16,
   which feeds the output projection (wo fp16 stationary).
 - AV+normalize for (w, h) is interleaved into the NEXT head's score loop;
   the output projection of window w-1 into the (w, h=1) loop; q/k/v
   projections + V transposes into the (0, 0) loop (x arrives in a
   fine-grained DMA ring).
"""

from contextlib import ExitStack

import numpy as np

HEADS = 8
DIM_HEAD = 64
SCALE = DIM_HEAD ** -0.5
B, N, D = 2, 4096, 512
NCORES = 8
E = 128          # inner-dim slice per core (2 heads)
JB = 128         # key block (contraction partition dim)
WI = 1024        # query i-window (et tile width)
SV = 68          # V free-dim stride per j-block (65 used)

_CACHE = {}


def _exp_poly_op():
    """Register (once) the custom DVE op EXP_POLY16_ANT:
    out = (1 + w + w^2*s0)^16 with w = in0 (pre-scaled scores), s0 = 1/2.
    This is exp(16*w) to ~7e-4 relative for |16*w| <= 1 (the score range);
    qT is pre-scaled by SCALE/16 so w = s*SCALE/16."""
    if "exp_op" in _CACHE:
        return _CACHE["exp_op"]
    import concourse.dve_ops as dve_ops
    from concourse.dve_ops import DveOp, OPS
    from concourse.dve_spec import Spec, Src0, C0, One, lower, sq
    from concourse.dve_spec import _has_src1
    from concourse.dve_uop import DveOpSpec

    name = "EXP_POLY16_ANT"
    if name not in dve_ops._SUB_OPCODE_FOR_NAME:
        p = (sq(Src0) * C0) + (Src0 + One)
        body = sq(sq(sq(sq(p))))
        spec = Spec(
            body=body,
            reference=lambda in0, in1, s0, s1, imm2: (
                1.0 + in0 + in0 * in0 * s0
            ) ** 16,
        )
        row = dve_ops._CUSTOM_DVE_ROW_BASE + len(OPS)
        assert row < 0x20, "custom-DVE row field overflow"
        shas = {}
        for ver in ("v3", "v4"):
            s_obj = DveOpSpec(name=name, opcode=row, uops=lower(spec, ver=ver),
                              rd1_en=_has_src1(spec))
            shas[ver] = s_obj.sha(ver)
        op = DveOp(name, spec, subdim=False, uops_sha=shas)
        OPS.append(op)
        dve_ops.CUSTOM_DVE_SPECS[name] = spec
        dve_ops._SUB_OPCODE_FOR_NAME[name] = row
    _CACHE["exp_op"] = next(o for o in OPS if o.name == name)
    return _CACHE["exp_op"]


# which j-blocks (mod 16) run exp on VectorE (custom DVE op) vs ScalarE
DVE_PAT = (0, 1, 0, 0, 1, 0, 0, 1, 0, 0, 1, 0, 0, 1, 0, 0)
DVE_PAT0 = (0, 0, 0, 0, 1, 0, 0, 0, 0, 0, 0, 0, 1, 0, 0, 0)


def build_program(n=N, wi_=None, st_bufs=2, o_bufs=2, pt_bufs=1, et_bufs=42,
                  x_bufs=5, av_per_step=32, dve_pat=DVE_PAT, dve_pat0=DVE_PAT0,
                  op_j0=8, op_dj=3, use_custom=1, norm_gpsimd=1):
    import concourse.bass as bass
    import concourse.tile as tile
    from concourse import bacc, mybir
    from concourse.masks import make_identity

    f32 = mybir.dt.float32
    f32r = mybir.dt.float32r
    bf16 = mybir.dt.bfloat16
    f16 = mybir.dt.float16
    Exp = mybir.ActivationFunctionType.Exp
    exp_op = _exp_poly_op() if use_custom else None

    wi = wi_ or WI
    nj = n // JB             # 128-key blocks
    nw = max(1, n // wi)
    nwc = wi // 512          # 512-chunks per window
    nib = wi // 128          # 128-row i-blocks per window
    nnb = n // 512           # 512-blocks over full seq

    nc = bacc.Bacc("TRN2", target_bir_lowering=False, debug=False,
                   num_devices=NCORES)

    xT = nc.dram_tensor("xT", [D, n], f32r, kind="ExternalInput").ap()
    wqT = nc.dram_tensor("wqT", [D, E], f32r, kind="ExternalInput").ap()
    wkT = nc.dram_tensor("wkT", [D, E], f32r, kind="ExternalInput").ap()
    wvT = nc.dram_tensor("wvT", [D, E], f32r, kind="ExternalInput").ap()
    woT = nc.dram_tensor("woT", [E, D], f16, kind="ExternalInput").ap()
    yT = nc.dram_tensor("yT", [D, n], f32, kind="ExternalOutput").ap()

    def emit_body(tc, ctx):
        const = ctx.enter_context(tc.tile_pool(name="const", bufs=1))
        persist = ctx.enter_context(tc.tile_pool(name="persist", bufs=1))
        xp = ctx.enter_context(tc.tile_pool(name="xp", bufs=x_bufs))
        etp = ctx.enter_context(tc.tile_pool(name="etp", bufs=et_bufs))
        psA = ctx.enter_context(
            tc.tile_pool(name="psA", bufs=st_bufs, space="PSUM"))
        psO = ctx.enter_context(
            tc.tile_pool(name="psO", bufs=o_bufs, space="PSUM"))
        # transposes (f16 [128,1024] = 1 bank) and outproj (f32 [128,512] =
        # 1 bank) share one ring so both get 2-deep pipelining in 2 banks
        psP = ctx.enter_context(
            tc.tile_pool(name="psP", bufs=pt_bufs, space="PSUM"))
        osbp = ctx.enter_context(tc.tile_pool(name="osbp", bufs=3))
        onpp = ctx.enter_context(tc.tile_pool(name="onpp", bufs=3))
        ysbp = ctx.enter_context(tc.tile_pool(name="ysb", bufs=4))

        identb = const.tile([128, 128], f16, name="identb", tag="identb")
        make_identity(nc, identb)

        # persistent SBUF tensors
        qT = persist.tile([E, n], bf16, name="qT", tag="qT")
        kT = persist.tile([E, n], bf16, name="kT", tag="kT")
        vT = persist.tile([E, n], f16, name="vT", tag="vT")
        OTn = persist.tile([E, n], f16, name="OTn", tag="OTn")
        V = [persist.tile([JB, nj, SV], f16, name=f"V{h}", tag=f"V{h}")
             for h in range(2)]
        wo_sb = persist.tile([E, D], f16, name="wo_sb", tag="wo_sb")

        # weights on the ACT DGE ring so they don't delay x on the SP ring
        wq_sb = persist.tile([128, 4, E], f32r, name="wq_sb", tag="wq_sb")
        wk_sb = persist.tile([128, 4, E], f32r, name="wk_sb", tag="wk_sb")
        wv_sb = persist.tile([128, 4, E], f32r, name="wv_sb", tag="wv_sb")
        nc.scalar.dma_start(out=wo_sb, in_=woT)
        for wsb, wdram in ((wq_sb, wqT), (wk_sb, wkT), (wv_sb, wvT)):
            for kc in range(4):
                nc.scalar.dma_start(out=wsb[:, kc, :],
                                    in_=wdram[kc * 128:(kc + 1) * 128, :])

        # ones column of V (disjoint region from the data columns)
        for h in range(2):
            nc.vector.memset(V[h][:, :, DIM_HEAD:DIM_HEAD + 1], 1.0)

        x_tiles = {}

        def x_dma(nb):
            t = xp.tile([128, 4, 512], f32r, name="xt", tag="xt")
            x_tiles[nb] = t
            for kc in range(4):
                nc.sync.dma_start(
                    out=t[:, kc, :],
                    in_=xT[kc * 128:(kc + 1) * 128, nb * 512:(nb + 1) * 512])

        def proj_sub(nb, which):
            """One projection (q, k or v) for one 512-wide block."""
            sl = slice(nb * 512, (nb + 1) * 512)
            wsb, dest = {"q": (wq_sb, qT), "k": (wk_sb, kT),
                         "v": (wv_sb, vT)}[which]
            ps = psO.tile([128, 512], f32, name="pp", tag="po")
            for kc in range(4):
                nc.tensor.matmul(
                    ps,
                    lhsT=wsb[:, kc, :],
                    rhs=x_tiles[nb][:, kc, :],
                    start=(kc == 0), stop=(kc == 3))
            if which == "q":
                # scores arrive pre-scaled by SCALE/16 (ScalarE exp uses
                # scale=16, the DVE poly needs no input multiply)
                nc.vector.tensor_scalar(
                    out=dest[:, sl], in0=ps,
                    scalar1=float(SCALE / 16.0), scalar2=None,
                    op0=mybir.AluOpType.mult)
            else:
                nc.vector.tensor_copy(dest[:, sl], ps)

        def proj_unit(nb):
            for which in ("q", "k", "v"):
                proj_sub(nb, which)

        def trans_unit(nb):
            """V natural fp16 layout for this block's 4 key blocks: four PE
            transposes into one PSUM tile, one strided copy per head."""
            tpt = psP.tile([128, 1024], f16, name="tpt", tag="pp")
            for jj in range(4):
                jb = nb * 4 + jj
                nc.tensor.transpose(tpt[:, jj * 128:(jj + 1) * 128],
                                    vT[:, jb * 128:(jb + 1) * 128], identb)
            # tpt free layout: (jj, h*64+d); V dest: (jb, d) strided SV
            src = tpt[:, 0:512].rearrange("p (jj h d) -> p jj h d", jj=4, h=2)
            for h in range(2):
                dst = V[h][:, nb * 4:(nb + 1) * 4, 0:DIM_HEAD]
                nc.vector.tensor_copy(dst, src[:, :, h:h + 1, :])

        def score_exp(w, h, j, pat, pat0):
            """Score matmuls + exp for one key block; returns et [j, i]."""
            e0, e1 = h * 64, (h + 1) * 64
            et = etp.tile([128, wi], f16, name="et", tag="et")
            st = psA.tile([128, wi], f32, name="st", tag="st")
            for c2 in range(nwc):
                i0 = w * wi + c2 * 512
                nc.tensor.matmul(
                    st[:, c2 * 512:(c2 + 1) * 512],
                    lhsT=kT[e0:e1, j * JB:(j + 1) * JB],
                    rhs=qT[e0:e1, i0:i0 + 512],
                    start=True, stop=True)
            use_pat = pat0 if (w == 0 and h == 0) else pat
            if exp_op is not None and use_pat[j % len(use_pat)]:
                nc.vector._custom_dve(exp_op, out=et, in0=st, s0=0.5)
            else:
                nc.scalar.activation(et, st, Exp, scale=16.0)
            return et

        def o_finish_a(O):
            """Evacuate + normalize one finished O i-block (DVE + gpsimd);
            the PE transpose is deferred (o_finish_b) so the PE queue never
            head-blocks on this chain."""
            osb = osbp.tile([128, 66], f32, name="osb", tag="osb")
            nc.vector.tensor_copy(osb[:, 0:DIM_HEAD + 1], O[:, 0:DIM_HEAD + 1])
            onp = onpp.tile([128, DIM_HEAD], f16, name="onp", tag="onp")
            if norm_gpsimd:
                nc.gpsimd.normalize_recip(
                    out_ap=onp, in_ap=osb[:, 0:DIM_HEAD],
                    denom_ap=osb[:, DIM_HEAD:DIM_HEAD + 1])
            else:
                rc = osbp.tile([128, 1], f32, name="rc", tag="rc")
                nc.vector.reciprocal(rc, osb[:, DIM_HEAD:DIM_HEAD + 1])
                nc.vector.tensor_scalar(
                    out=onp, in0=osb[:, 0:DIM_HEAD], scalar1=rc,
                    scalar2=None, op0=mybir.AluOpType.mult)
            return onp

        def o_finish_b(hp, wp, ib, onp, pt_state):
            """Transpose a normalized i-block into OTn (PE + DVE copy)."""
            if ib % 4 == 0:
                pt_state["t"] = psP.tile([128, 1024], f16, name="ot", tag="pp")
            tpt = pt_state["t"]
            nc.tensor.transpose(
                tpt[0:DIM_HEAD, (ib % 4) * 128:(ib % 4 + 1) * 128], onp,
                identb)
            if ib % 4 == 3:
                half = ib // 4
                nc.vector.tensor_copy(
                    OTn[hp * 64:(hp + 1) * 64,
                        wp * wi + half * 512:wp * wi + (half + 1) * 512],
                    tpt[0:DIM_HEAD, 0:512])

        def outproj_unit(w, k, tail=False):
            """One of the 8 output-projection blocks of window w."""
            ib = w * nwc + k // 4
            dc = k % 4
            ps2 = psP.tile([128, 512], f32, name="ps2", tag="ps2")
            nc.tensor.matmul(
                ps2,
                lhsT=wo_sb[:, dc * 128:(dc + 1) * 128],
                rhs=OTn[:, ib * 512:(ib + 1) * 512],
                start=True, stop=True)
            yt = ysbp.tile([128, 512], f32, name="yt", tag="yt")
            # in the tail ScalarE is idle; split the evacuation
            if tail and k % 2 == 0:
                nc.scalar.copy(yt, ps2)
            else:
                nc.vector.tensor_copy(yt, ps2)
            nc.sync.dma_start(
                out=yT[dc * 128:(dc + 1) * 128,
                       ib * 512:(ib + 1) * 512],
                in_=yt)

        ETS = {}

        def make_av_fill(hp, wp, outproj_w=None):
            """fill(j) that interleaves AV+finish of (hp, wp) and optionally
            the output projection of window outproj_w."""
            ets = ETS.pop((wp, hp))
            O_state = {}
            g = [0]
            done_op = [0]
            pending_b = []

            def fill(j):
                take = min(av_per_step, nib * nj - g[0])
                for _ in range(take):
                    ib, jj = divmod(g[0], nj)
                    if jj == 0:
                        O_state["O"] = psO.tile([128, 512], f32,
                                                name="O", tag="po")
                    nc.tensor.matmul(
                        O_state["O"][:, 0:DIM_HEAD + 1],
                        lhsT=ets[jj][:, ib * 128:(ib + 1) * 128],
                        rhs=V[hp][:, jj, 0:DIM_HEAD + 1],
                        start=(jj == 0), stop=(jj == nj - 1))
                    g[0] += 1
                    if jj == nj - 1:
                        pending_b.append((ib, o_finish_a(O_state["O"])))
                # transpose lags the normalize chain by ~2 i-blocks so the
                # PE queue never waits on DVE/gpsimd; flush rate 1 per step
                if pending_b and (g[0] >= (pending_b[0][0] + 2) * nj
                                  or g[0] >= nib * nj):
                    ib, onp = pending_b.pop(0)
                    o_finish_b(hp, wp, ib, onp, O_state)
                if (outproj_w is not None and j >= op_j0
                        and (j - op_j0) % op_dj == 0 and done_op[0] < nwc * 4):
                    outproj_unit(outproj_w, done_op[0])
                    done_op[0] += 1

            return fill

        def head_loop(w, h, fill):
            ets = []
            for j in range(nj):
                ets.append(score_exp(w, h, j, dve_pat, dve_pat0))
                fill(j)
            ETS[(w, h)] = ets

        # ---- (0, 0): projections + V transposes fill the score loop ----
        for nb in range(min(4, nnb)):
            x_dma(nb)
        proj_unit(0)
        proj_unit(1)

        def fill_w0h0(j):
            nb = j // 4 + 2
            if j % 4 == 0 and j // 4 + 4 < nnb:
                x_dma(j // 4 + 4)
            if j % 4 == 1 and nb < nnb:
                proj_sub(nb, "q")
            elif j % 4 == 2 and nb < nnb:
                proj_sub(nb, "k")
            elif j % 4 == 3 and nb < nnb:
                proj_sub(nb, "v")
            if j % 4 == 1:
                k = (j + 3) // 4  # trans_unit k at j = 4k-3
                if k <= nnb - 1:
                    trans_unit(k)
            if j == 0:
                trans_unit(0)

        head_loop(0, 0, fill_w0h0)

        # ---- remaining loops: AV of the previous (h, w) interleaves ----
        seq = [(w, h) for w in range(nw) for h in range(2)]
        for idx in range(1, len(seq)):
            w, h = seq[idx]
            wp, hp = seq[idx - 1]
            opw = w - 1 if (h == 1 and w >= 1) else None
            head_loop(w, h, make_av_fill(hp, wp, outproj_w=opw))

        # ---- tail: AV of the last loop + final output projection ----
        wp, hp = seq[-1]
        ets = ETS.pop((wp, hp))
        pt_state = {}
        pending_b = []
        for ib in range(nib):
            O = psO.tile([128, 512], f32, name="O", tag="po")
            for jj in range(nj):
                nc.tensor.matmul(
                    O[:, 0:DIM_HEAD + 1],
                    lhsT=ets[jj][:, ib * 128:(ib + 1) * 128],
                    rhs=V[hp][:, jj, 0:DIM_HEAD + 1],
                    start=(jj == 0), stop=(jj == nj - 1))
            pending_b.append((ib, o_finish_a(O)))
            if len(pending_b) > 1:
                ibb, onp = pending_b.pop(0)
                o_finish_b(hp, wp, ibb, onp, pt_state)
        for ibb, onp in pending_b:
            o_finish_b(hp, wp, ibb, onp, pt_state)
        for k in range(nwc * 4):
            outproj_unit(nw - 1, k, tail=True)

    with tile.TileContext(nc) as tc:
        with ExitStack() as ctx:
            emit_body(tc, ctx)

    nc.compile()
    return nc


def make_in_maps(x, Wq, Wk, Wv, Wo):
    x = np.asarray(x, np.float32)
    Wq = np.asarray(Wq, np.float32)
    Wk = np.asarray(Wk, np.float32)
    Wv = np.asarray(Wv, np.float32)
    Wo = np.asarray(Wo, np.float32)
    in_maps = []
    for c in range(NCORES):
        b, p = divmod(c, NCORES // B)
        e0 = p * E
        in_maps.append({
            "xT": np.ascontiguousarray(x[b].T),
            "wqT": np.ascontiguousarray(Wq.T[:, e0:e0 + E]),
            "wkT": np.ascontiguousarray(Wk.T[:, e0:e0 + E]),
            "wvT": np.ascontiguousarray(Wv.T[:, e0:e0 + E]),
            "woT": np.ascontiguousarray(Wo.T[e0:e0 + E, :]).astype(np.float16),
        })
    return in_maps


LAST_RESULTS = None


def kernel(x, Wq, Wk, Wv, Wo, bo):
    global LAST_RESULTS
    from concourse.bass_utils import run_bass_kernel_spmd

    if "nc" not in _CACHE:
        _CACHE["nc"] = build_program()
    nc = _CACHE["nc"]

    in_maps = make_in_maps(x, Wq, Wk, Wv, Wo)
    res = run_bass_kernel_spmd(nc, in_maps, core_ids=list(range(NCORES)))
    LAST_RESULTS = res

    y = np.zeros((B, N, D), np.float32)
    for c in range(NCORES):
        b = c // (NCORES // B)
        y[b] += res.results[c]["yT"].T
    y += np.asarray(bo, np.float32)
    return y
